# revision 1
# baseline (speedup 1.0000x reference)
"""AllAtomFAPE loss kernel for Trainium2 (8 NeuronCores, SPMD).

Problem: b=1, N=384 res, F=8 frames/res -> NF=3072 frames; A=14 atoms/res
-> NA=5376 atoms. Output: scalar (shape (1,)) masked clamped FAPE.

Algorithm (factorized pairwise distance):
  With P = pR pR^T, T = tR tR^T, M = pR tR^T (per frame, 3x3),
    d2(f,a) = (pp-pt)^T P (pp-pt) + (tp-tt)^T T (tp-tt)
              - 2 (pp-pt)^T M (tp-tt)
  expands into a K=34 dot product between a frame feature vector W[:,f]
  and an atom feature vector Z[:,a]:
    rows 0-8   : P[i,j]            <->  pp_i pp_j
    rows 9-17  : T[i,j]            <->  tp_i tp_j
    rows 18-26 : M[i,j]            <->  -2 pp_i tp_j
    rows 27-29 : 2(M tt - P pt)    <->  pp
    rows 30-32 : 2(M^T pt - T tt)  <->  tp
    row  33    : c_f               <->  1
  so the whole pairwise computation is one (34 x NF) x (34 x NA) matmul
  on the TensorEngine (bf16). Then (ScalarE) d = m_a*sqrt(d2+eps) via
  sqrt(scale*x+bias) with per-partition scale=m^2, bias=m^2*eps, and
  (VectorE) fused clamp+reduce: tensor_scalar(min thr=10*m_a, accum add).

Sharding: atoms sharded across the 8 cores (672 each, padded to 768);
frames replicated. Each core emits one partial scalar; the host sums
the 8 partials (the gather/unshard step).

Layouts: frame f = 24*p + t (partition p, block t); slabs are
row-major in the feature index r with the block index t innermost
(unit stride) so DVE ops hit the packed 2x bf16 mode. Feature slabs
are transposed to [34, entity] via PE transposes (3 blocks packed per
transpose -> [102, 128] in PSUM) + strided DVE copies.
"""

import numpy as np

import concourse.bacc as bacc
import concourse.bass as bass
import concourse.tile as tile
from concourse import mybir
from concourse.bass_utils import run_bass_kernel_spmd

F32 = mybir.dt.float32
BF16 = mybir.dt.bfloat16
AX = mybir.AxisListType
OP = mybir.AluOpType
ACTF = mybir.ActivationFunctionType

NCORES = 8
NF = 3072          # frames (N*F)
TFB = 24           # frame blocks per partition (f = 24*p + t)
NA = 5376          # atoms (N*A)
NAS = NA // NCORES  # 672 atoms per core
NAPAD = 768        # padded per-core atoms
TAB = 6            # atom blocks per partition (a = 6*p + t)
K = 34             # feature dim
KS = 64            # stored feature rows (zero-padded, 128/KS-aligned for
                   # the DMA xbar transpose + 32-aligned strip copies)
CH = 1536          # frame chunk (PSUM cols) per ACT/DVE op
NCH = NF // CH     # 2 chunks
MMN = 512          # matmul moving free dim
X_FUSED = 12       # chunks (of 12) whose clamp+reduce is fused on DVE at 1x
                   # (measured cheapest reduction: ACT Copy+accum ~2.4us/chunk
                   # and PE ones-matmuls both regressed).
EPS = 1e-4
EPS_EFF = EPS      # sqrt(neg)=NaN is filtered by the DVE min (min(NaN,thr)=thr,
                   # verified on HW); rare tiny-d2 pairs hitting that path
                   # contribute ~1e-4 relative error.
CLAMP = 10.0
ZSCALE = 10.0
CNORM = float(1.0 / (ZSCALE * (3072.0 + EPS)))


def _bc(ap, dim, n):
    """Broadcast AP along a new axis at position `dim` (stride-0), n copies."""
    return ap.unsqueeze(dim).to_broadcast(
        tuple(ap.shape[:dim]) + (n,) + tuple(ap.shape[dim:])
    )


def build_nc():
    nc = bacc.Bacc(None)

    # inputs consolidated into two params -> two DMAs (queue latency is
    # ~2us per dma_start; nine separate loads stalled the feature ops)
    FRW = 2 * 9 * TFB + 2 * 3 * TFB          # pr|tr|pt|tt = 576
    ATW = 2 * 3 * TAB + TAB + NA // 128 + 128  # pp|tp|am|amf|ident = 212
    fr_d = nc.declare_dram_parameter("fr", [128, FRW], F32, isOutput=False)
    at_d = nc.declare_dram_parameter("at", [128, ATW], F32, isOutput=False)
    out_d = nc.declare_dram_parameter("out", [1, 2], F32, isOutput=True)

    with tile.TileContext(nc) as tc:
        with (
            tc.tile_pool(name="consts", bufs=1) as consts,
            tc.tile_pool(name="feat", bufs=1) as feat,
            tc.tile_pool(name="psum", bufs=2, space="PSUM") as psum_pool,
            tc.tile_pool(name="sbuf_s", bufs=3) as sbuf_s,
        ):
            # ---------------- input DMAs (two consolidated loads) --------
            frs = consts.tile([128, FRW], F32)
            ats = consts.tile([128, ATW], F32)
            nc.sync.dma_start(out=frs[:], in_=fr_d[:])
            nc.scalar.dma_start(out=ats[:], in_=at_d[:])
            fr_ap = frs[:]
            pRs = fr_ap[:, 0:216]
            tRs = fr_ap[:, 216:432]
            pts = fr_ap[:, 432:504]
            tts = fr_ap[:, 504:576]
            at_ap = ats[:]
            pps = at_ap[:, 0:18]
            tps = at_ap[:, 18:36]
            ams = at_ap[:, 36:42]
            amf = at_ap[:, 42:84]
            identf = at_ap[:, 84:212]
            identity = consts.tile([128, 128], BF16)
            nc.vector.tensor_copy(identity[:], identf)
            pRb, tRb, ptb, ttb, ppb, tpb = pRs, tRs, pts, tts, pps, tps

            # ------------- frame features (fp32, two t-halves) ----------
            # Slab col = KS*t + r. Computed per t-half so the first half's
            # transposes + main-loop chunk ch=0 overlap the second half.
            Wslab = feat.tile([128, KS * TFB], F32)
            nc.vector.memset(Wslab[:], 0.0)
            TH = TFB // 2
            mul0 = feat.tile([128, 9 * TH], F32)
            mul1 = feat.tile([128, 9 * TH], F32)
            mul2 = feat.tile([128, 9 * TH], F32)
            tmp3b = feat.tile([128, 3 * TH], F32)
            tmp3 = feat.tile([128, 3 * TFB], F32)
            tmp1 = feat.tile([128, TFB], F32)
            Ppt = feat.tile([128, 3 * TFB], F32)
            Mtt = feat.tile([128, 3 * TFB], F32)
            Ttt = feat.tile([128, 3 * TFB], F32)
            Mtp = feat.tile([128, 3 * TFB], F32)

            W4a = Wslab[:].rearrange("p (t r) -> p r t", r=KS)         # [128,64,24]
            R4a = pRb.rearrange("p (c t) -> p c t", c=9)
            T4a = tRb.rearrange("p (c t) -> p c t", c=9)
            pt3a = ptb.rearrange("p (c t) -> p c t", c=3)           # [128,3,24]
            tt3a = ttb.rearrange("p (c t) -> p c t", c=3)
            m0v = mul0[:].rearrange("p (i j t) -> p i j t", i=3, j=3)  # contig
            m1v = mul1[:].rearrange("p (i j t) -> p i j t", i=3, j=3)
            m2v = mul2[:].rearrange("p (i j t) -> p i j t", i=3, j=3)
            t3bv = tmp3b[:].rearrange("p (c t) -> p c t", c=3)

            def frame_feats(lo, hi):
                W4 = W4a[:, :, lo:hi]
                R4 = R4a[:, :, lo:hi].rearrange("p (i k) t -> p i k t", i=3)
                T4 = T4a[:, :, lo:hi].rearrange("p (i k) t -> p i k t", i=3)
                pt3 = pt3a[:, :, lo:hi]
                tt3 = tt3a[:, :, lo:hi]

                def gram(out4, A4, B4):
                    # 3 muls to contiguous temps, adds; only the last add
                    # writes the (KS-strided) slab rows.
                    a = lambda k: _bc(A4[:, :, k, :], 2, 3)
                    b = lambda k: _bc(B4[:, :, k, :], 1, 3)
                    nc.vector.tensor_mul(m0v, a(0), b(0))
                    nc.vector.tensor_mul(m1v, a(1), b(1))
                    nc.vector.tensor_mul(m2v, a(2), b(2))
                    nc.vector.tensor_add(m0v, m0v, m1v)
                    nc.vector.tensor_add(out4, m0v, m2v)

                Pv = W4[:, 0:9, :].rearrange("p (i j) t -> p i j t", i=3)
                Tv = W4[:, 9:18, :].rearrange("p (i j) t -> p i j t", i=3)
                Mv = W4[:, 18:27, :].rearrange("p (i j) t -> p i j t", i=3)
                gram(Pv, R4, R4)
                gram(Tv, T4, T4)
                gram(Mv, R4, T4)

                mjit = mul0[:].rearrange("p (j i t) -> p j i t", j=3, i=3)

                def matvec(out3, Q, vec3, transpose=False):
                    # one wide mul over (j,i,t), then two adds
                    qv = Q.transpose([0, 2, 1, 3]) if not transpose else Q
                    mj = mjit[:, :, :, 0:vec3.shape[2]]
                    nc.vector.tensor_mul(mj, qv, _bc(vec3, 2, 3))
                    nc.vector.tensor_add(t3bv, mj[:, 0, :, :], mj[:, 1, :, :])
                    nc.vector.tensor_add(out3, t3bv, mj[:, 2, :, :])

                Ppt3 = Ppt[:].rearrange("p (c t) -> p c t", c=3)[:, :, lo:hi]
                Mtt3 = Mtt[:].rearrange("p (c t) -> p c t", c=3)[:, :, lo:hi]
                Ttt3 = Ttt[:].rearrange("p (c t) -> p c t", c=3)[:, :, lo:hi]
                Mtp3 = Mtp[:].rearrange("p (c t) -> p c t", c=3)[:, :, lo:hi]
                matvec(Ppt3, Pv, pt3)
                matvec(Mtt3, Mv, tt3)
                matvec(Ttt3, Tv, tt3)
                matvec(Mtp3, Mv, pt3, transpose=True)  # M^T pt

                tmp3v = tmp3[:].rearrange("p (c t) -> p c t", c=3)[:, :, lo:hi]
                nc.vector.tensor_sub(tmp3v, Mtt3, Ppt3)
                nc.vector.tensor_scalar_mul(W4[:, 27:30, :], tmp3v, 2.0)
                nc.vector.tensor_sub(tmp3v, Mtp3, Ttt3)
                nc.vector.tensor_scalar_mul(W4[:, 30:33, :], tmp3v, 2.0)

                # cf row 33: pt.(Ppt - 2*Mtt) + tt.Ttt
                cfb = W4[:, 33, :]
                t1b = tmp1[:, lo:hi]
                nc.vector.tensor_sub(tmp3v, Ppt3, Mtt3)
                nc.vector.tensor_sub(tmp3v, tmp3v, Mtt3)
                # dot products via one wide mul each, then pairwise adds
                pd = t3bv  # [128,3,TH]
                nc.vector.tensor_mul(pd, tmp3v, pt3)
                nc.vector.tensor_add(t1b, pd[:, 0, :], pd[:, 1, :])
                nc.vector.tensor_add(cfb, t1b, pd[:, 2, :])
                nc.vector.tensor_mul(pd, Ttt3, tt3)
                nc.vector.tensor_add(t1b, pd[:, 0, :], pd[:, 1, :])
                nc.vector.tensor_add(t1b, t1b, pd[:, 2, :])
                nc.vector.tensor_add(cfb, cfb, t1b)

            # ------------- atom features (sharded) ----------------------
            Zslab = feat.tile([128, KS * TAB], F32)
            nc.vector.memset(Zslab[:], 0.0)
            Z4 = Zslab[:].rearrange("p (t r) -> p r t", r=KS)          # [128,34,6]
            pp3 = ppb.rearrange("p (c t) -> p c t", c=3)           # [128,3,6]
            tp3 = tpb.rearrange("p (c t) -> p c t", c=3)
            n2pp = feat.tile([128, 3 * TAB], F32)
            nc.vector.tensor_scalar_mul(n2pp[:], ppb, -2.0)
            n2pp3 = n2pp[:].rearrange("p (c t) -> p c t", c=3)

            Zpp = Z4[:, 0:9, :].rearrange("p (i j) t -> p i j t", i=3)
            Ztp = Z4[:, 9:18, :].rearrange("p (i j) t -> p i j t", i=3)
            Zx = Z4[:, 18:27, :].rearrange("p (i j) t -> p i j t", i=3)
            nc.vector.tensor_mul(Zpp, _bc(pp3, 2, 3), _bc(pp3, 1, 3))
            nc.vector.tensor_mul(Ztp, _bc(tp3, 2, 3), _bc(tp3, 1, 3))
            nc.vector.tensor_mul(Zx, _bc(n2pp3, 2, 3), _bc(tp3, 1, 3))
            nc.vector.tensor_copy(Z4[:, 27:30, :], pp3)
            nc.vector.tensor_copy(Z4[:, 30:33, :], tp3)
            nc.vector.memset(Z4[:, 33, :], 1.0)

            # mask-derived per-partition vectors (fp32)
            scale_v = consts.tile([128, TAB], F32)   # m^2
            bias_v = consts.tile([128, TAB], F32)    # m^2 * eps_eff
            thr_v = consts.tile([128, TAB], F32)     # 10 * m
            nc.vector.tensor_mul(scale_v[:], ams, ams)
            nc.vector.tensor_scalar_mul(bias_v[:], scale_v[:], EPS_EFF)
            nc.vector.tensor_scalar_mul(thr_v[:], ams, CLAMP)

            # ------------- transposes (PE, 128-col groups = 2 blocks) ---
            Wslab_b = feat.tile([128, KS * TFB], BF16)
            Zslab_b = feat.tile([128, KS * TAB], BF16)
            nc.vector.tensor_copy(Zslab_b[:], Zslab[:])
            NGW = KS * TFB // 128   # 12 groups
            NGZ = KS * TAB // 128   # 3 groups
            WT = consts.tile([KS, NF], BF16)
            ZT = consts.tile([KS, NAPAD], BF16)
            WT5 = WT[:].rearrange("q (g s c) -> q g s c", g=NGW, s=2)
            ZT5 = ZT[:].rearrange("q (g s c) -> q g s c", g=NGZ, s=2)

            def w_transpose_half(half):
                # cast this half of the slab, then 6 groups via PE
                HC = KS * TFB // 2
                nc.vector.tensor_copy(
                    Wslab_b[:, HC * half:HC * (half + 1)],
                    Wslab[:, HC * half:HC * (half + 1)])
                for q in range(3):
                    pst = psum_pool.tile([128, 512], BF16, tag="tp")
                    for u in range(2):
                        g = 6 * half + 2 * q + u
                        nc.tensor.transpose(
                            pst[:, 128 * u:128 * (u + 1)],
                            Wslab_b[:, 128 * g:128 * (g + 1)],
                            identity[:])
                    pst3 = pst[:, 0:256].rearrange("q (u c) -> q u c", c=128)
                    for s in range(2):
                        nc.vector.tensor_copy(
                            WT5[:, 6 * half + 2 * q:6 * half + 2 * q + 2, s, :],
                            pst3[64 * s:64 * (s + 1), :, :])

            pstz = psum_pool.tile([128, 512], BF16, tag="tp")
            for g in range(NGZ):
                nc.tensor.transpose(
                    pstz[:, 128 * g:128 * (g + 1)],
                    Zslab_b[:, 128 * g:128 * (g + 1)],
                    identity[:])
            pstz3 = pstz[:, 0:128 * NGZ].rearrange("q (u c) -> q u c", c=128)
            for s in range(2):
                nc.vector.tensor_copy(
                    ZT5[:, :, s, :], pstz3[64 * s:64 * (s + 1), :, :])


            frame_feats(0, TFB // 2)
            w_transpose_half(0)
            frame_feats(TFB // 2, TFB)
            w_transpose_half(1)

            # ------------- main loop ------------------------------------
            colacc = consts.tile([128, TAB * NCH], F32)
            scratch = consts.tile([128, CH], BF16)
            ones_b = consts.tile([128, 128], BF16)
            nc.vector.memset(ones_b[:], 1.0)

            pe_chunks = []
            idx = 0
            first_red = [True]
            for ch in range(NCH):
                for a in range(TAB):
                    zt = ZT[:, 128 * a:128 * (a + 1)]
                    ps = psum_pool.tile([128, CH], F32, tag="main")
                    for m in range(CH // MMN):
                        col = ch * CH + m * MMN
                        nc.tensor.matmul(
                            ps[:, m * MMN:(m + 1) * MMN],
                            zt,
                            WT[:, col:col + MMN],
                        )
                    s = sbuf_s.tile([128, CH], BF16)
                    nc.scalar.activation(
                        out=s[:],
                        in_=ps[:],
                        func=ACTF.Sqrt,
                        bias=bias_v[:, a:a + 1],
                        scale=scale_v[:, a:a + 1],
                    )
                    if (idx * X_FUSED) % 12 < X_FUSED and X_FUSED > 0:
                        nc.vector.tensor_scalar(
                            out=scratch[:],
                            in0=s[:],
                            scalar1=thr_v[:, a:a + 1],
                            scalar2=None,
                            op0=OP.min,
                            op1=OP.add,
                            accum_out=colacc[:, idx:idx + 1],
                        )
                    else:
                        # min at 4x on DVE (also filters sqrt-NaNs), then
                        # ScalarE sums the clamped tile via Copy+accum_out.
                        d = sbuf_s.tile([128, CH], BF16, tag="dmin")
                        nc.vector.tensor_scalar(
                            out=d[:],
                            in0=s[:],
                            scalar1=thr_v[:, a:a + 1],
                            scalar2=None,
                            op0=OP.min,
                        )
                        nc.scalar.activation(
                            out=scratch[:],
                            in_=d[:],
                            func=ACTF.Copy,
                            accum_out=colacc[:, idx:idx + 1],
                        )
                    idx += 1

            # ------------- epilogue -------------------------------------
            Sc = consts.tile([128, 1], F32)
            Sc2 = consts.tile([128, 1], F32)
            Mc = consts.tile([128, 1], F32)
            nc.vector.reduce_sum(out=Sc[:], in_=colacc[:], axis=AX.X)
            nc.vector.reduce_sum(out=Mc[:], in_=amf, axis=AX.X)
            ones_f = consts.tile([128, 1], F32)
            nc.vector.memset(ones_f[:], 1.0)
            psfin = psum_pool.tile([1, 2], F32, tag="tp")
            nc.tensor.matmul(psfin[:, 0:1], Sc[:], ones_f[:])
            nc.tensor.matmul(psfin[:, 1:2], Mc[:], ones_f[:])
            t0 = consts.tile([1, 1], F32)
            t1 = consts.tile([1, 1], F32)
            res = consts.tile([1, 2], F32)
            nc.vector.tensor_scalar(
                out=t0[:], in0=psfin[0:1, 1:2], scalar1=EPS, scalar2=None, op0=OP.add
            )
            nc.vector.reciprocal(t1[:], t0[:])
            nc.vector.tensor_scalar(
                out=res[:, 0:1], in0=psfin[0:1, 0:1], scalar1=t1[0:1, 0:1],
                scalar2=CNORM, op0=OP.mult, op1=OP.mult,
            )
            nc.vector.tensor_copy(res[:, 1:2], t0[:])
            nc.sync.dma_start(out=out_d[:], in_=res[:])

    nc.compile()
    return nc


def prep_in_maps(inputs):
    """Full (unsharded) numpy inputs -> per-core input dicts.

    Component-major SBUF layouts: frame f = 24*p + t lives at partition p,
    block t; a [*, C]-component tensor becomes [128, C*TFB] with column
    c*TFB + t. Atoms: a = 6*p + t, padded 672 -> 768 with zeros.
    """
    f32 = np.float32

    def fr(x, comps):
        return np.ascontiguousarray(
            np.asarray(x, f32).reshape(128, TFB, comps).transpose(0, 2, 1)
        ).reshape(128, comps * TFB)

    def at(x, comps, c):
        buf = np.zeros((NAPAD, comps), f32)
        buf[:NAS] = np.asarray(x, f32).reshape(NA, comps)[c * NAS:(c + 1) * NAS]
        return np.ascontiguousarray(
            buf.reshape(128, TAB, comps).transpose(0, 2, 1)
        ).reshape(128, comps * TAB)

    pR = fr(inputs["predicted_frames_R"], 9)
    tR = fr(inputs["true_frames_R"], 9)
    pt = fr(inputs["predicted_frames_t"], 3)
    tt = fr(inputs["true_frames_t"], 3)
    am_flat = np.asarray(inputs["atom_mask"], f32).reshape(NA)
    amf = np.ascontiguousarray(am_flat).reshape(128, NA // 128)

    fr = np.ascontiguousarray(np.concatenate([pR, tR, pt, tt], axis=1))
    ident = np.eye(128, dtype=f32)
    in_maps = []
    for c in range(NCORES):
        amp = np.zeros((NAPAD,), f32)
        amp[:NAS] = am_flat[c * NAS:(c + 1) * NAS]
        atc = np.ascontiguousarray(np.concatenate([
            at(inputs["predicted_atom_positions"], 3, c),
            at(inputs["true_atom_positions"], 3, c),
            amp.reshape(128, TAB),
            amf,
            ident,
        ], axis=1))
        in_maps.append({"fr": fr, "at": atc})
    return in_maps


_NC_CACHE = None


def _get_nc():
    global _NC_CACHE
    if _NC_CACHE is None:
        _NC_CACHE = build_nc()
    return _NC_CACHE


def kernel(**inputs):
    nc = _get_nc()
    in_maps = prep_in_maps(inputs)
    r = run_bass_kernel_spmd(nc, in_maps, core_ids=list(range(NCORES)))
    total = np.float32(0.0)
    for i in range(NCORES):
        total += np.float32(r.results[i]["out"][0, 0])
    return np.array([total], dtype=np.float32)



# revision 15
# speedup vs baseline: 1.1037x; 1.1037x over previous
"""AllAtomFAPE loss kernel for Trainium2 (8 NeuronCores, SPMD) — v2.

Problem: b=1, N=384 res, F=8 frames/res -> NF=3072 frames; A=14 atoms/res
-> NA=5376 atoms. Output: scalar masked clamped FAPE.

Algorithm (factorized pairwise distance):
  With P = pR pR^T, T = tR tR^T, M = pR tR^T (per frame, 3x3),
    d2(f,a) = (pp-pt)^T P (pp-pt) + (tp-tt)^T T (tp-tt)
              - 2 (pp-pt)^T M (tp-tt)
  expands into a K=39 dot product (rows padded to even offsets so the
  bf16 atom-feature writes stay 4B-aligned; pad rows are zero on both
  sides) between frame features W[:,f] and atom features Z[:,a]:
    rows 0-8   : P[i,j]          <->  pp_i pp_j
    rows 10-18 : M[i,j]          <->  -2 pp_i tp_j
    rows 20-28 : T[i,j]          <->  tp_i tp_j
    rows 30-32 : (M tt - P pt)   <->  2 pp
    rows 34-36 : (M^T pt - T tt) <->  2 tp
    row  38    : c_f + EPS       <->  m_a^2   (mask + eps folded in, so
                                      PSUM d2 = m^2 (d2_true + eps))
  ScalarE computes d = sqrt(psum) with no scale/bias, DVE clamps with a
  constant min(d, 10) (exact for binary masks: m=0 -> d=0), and sums go
  through PE ones-matmuls (most tiles) or the fused DVE min+accum.

Sharding: FRAMES sharded across the 8 cores (384 frames = 3 partition
blocks each, no padding); atoms replicated (5376 = 42 blocks as the
moving free dim). Each core emits [S_partial, mask_sum]; the host does
S_total * CNORM / (eps + mask_sum).

Layouts: PE-transposed via identity matmuls.  W slab holds each frame
block duplicated into both 64-row slots of its group (col = 128*g +
64*s + r) so a stationary exists at base partition 0 AND 64; the
stationary for (fb, u) is WTi[64*u + (0:39), 128*fb + (0:128)].
Atom slab uses pairing (g, g+21): col = 128*g + 64*u + r with block
t = g + 21*u, so ZTi[64*u + (0:39), :] is a contiguous [39, 2688]
moving operand for atom half u.
"""

import numpy as np

import concourse.bacc as bacc
import concourse.bass as bass
import concourse.tile as tile
from concourse import mybir
from concourse.bass_utils import run_bass_kernel_spmd

F32 = mybir.dt.float32
BF16 = mybir.dt.bfloat16
AX = mybir.AxisListType
OP = mybir.AluOpType
ACTF = mybir.ActivationFunctionType

NCORES = 8
NF = 3072
NFC = NF // NCORES     # 384 frames per core
NFB = NFC // 128       # 3 frame blocks
NA = 5376
NAB = NA // 128        # 42 atom blocks
GZ = NAB // 2          # 21 atom groups
K = 39                 # contract rows (with pads)
KS = 64
CHA = 1344             # atom cols per main tile (4 tiles per frame block)
NCH = NA // CHA        # 4
NT = NFB * NCH         # 12 main tiles
EPS = 1e-4
CLAMP = 10.0
ZSCALE = 10.0
CNORM = float(1.0 / (ZSCALE * (3072.0 + EPS)))

# tile indices whose clamp+sum uses the fused DVE min+accum (1x DVE);
# the rest use DVE min (4x) + a delayed PE ones-matmul reduction.
FUSED = (9, 10, 11)
ONES_DELAY = 2         # ones-matmul for tile i issues after mm of tile i+2


def _bc(ap, dim, n):
    """Broadcast AP along a new axis at position `dim` (stride-0), n copies."""
    return ap.unsqueeze(dim).to_broadcast(
        tuple(ap.shape[:dim]) + (n,) + tuple(ap.shape[dim:])
    )


def build_nc():
    nc = bacc.Bacc(None)

    FRW = 9 * NFB * 2 + 3 * NFB * 2          # 72 cols fp32
    ATW = 3 * NAB * 2 + NAB + 128            # 422 cols bf16 (incl identity)
    fr_d = nc.declare_dram_parameter("fr", [128, FRW], F32, isOutput=False)
    at_d = nc.declare_dram_parameter("at", [128, ATW], BF16, isOutput=False)
    out_d = nc.declare_dram_parameter("out", [2, 1], F32, isOutput=True)

    with tile.TileContext(nc) as tc:
        with (
            tc.tile_pool(name="consts", bufs=1) as consts,
            tc.tile_pool(name="sbuf_s", bufs=3) as sbuf_s,
        ):
            # ---------------- input DMAs --------------------------------
            ats = consts.tile([128, ATW], BF16)
            frs = consts.tile([128, FRW], F32)
            nc.sync.dma_start(out=ats[:], in_=at_d[:])
            nc.gpsimd.dma_start(out=frs[:], in_=fr_d[:])
            at_ap = ats[:]
            ppb = at_ap[:, 0:126]                  # col = c*42 + t
            tpb = at_ap[:, 126:252]
            amb = at_ap[:, 252:294]
            ident = at_ap[:, 294:422]
            fr_ap = frs[:]
            ptb = fr_ap[:, 54:63]                  # col = 54 + 3c + t
            ttb = fr_ap[:, 63:72]

            # ---------------- atom (Z) features, bf16 -------------------
            Zslab = consts.tile([128, 128 * GZ], BF16)
            nc.vector.memset(Zslab[:], 0.0)
            n2pp = consts.tile([128, 126], BF16)
            nc.vector.tensor_scalar_mul(n2pp[:], ppb, -2.0)

            pp3 = ppb.rearrange("p (c t) -> p c t", c=3)       # [128,3,42]
            tp3 = tpb.rearrange("p (c t) -> p c t", c=3)
            n2pp3 = n2pp[:].rearrange("p (c t) -> p c t", c=3)
            Zg = Zslab[:].rearrange("p (g v) -> p g v", v=128)  # [128,21,128]

            def z_feats(u):
                lo, hi = GZ * u, GZ * u + GZ
                b0 = 64 * u
                # [p, g, c] operand views (transpose puts g before c)
                ppu = pp3[:, :, lo:hi].transpose([0, 2, 1])
                tpu = tp3[:, :, lo:hi].transpose([0, 2, 1])
                n2u = n2pp3[:, :, lo:hi].transpose([0, 2, 1])

                def quad(r0, av, bv):
                    out = Zg[:, :, b0 + r0:b0 + r0 + 9].rearrange(
                        "p g (i j) -> p g i j", i=3)
                    nc.vector.tensor_mul(out, _bc(av, 3, 3), _bc(bv, 2, 3))

                quad(0, ppu, ppu)     # pp_i pp_j
                quad(10, n2u, tpu)    # -2 pp_i tp_j
                quad(20, tpu, tpu)    # tp_i tp_j
                nc.vector.tensor_scalar_mul(
                    Zg[:, :, b0 + 30:b0 + 33], ppu, 2.0)
                nc.vector.tensor_scalar_mul(
                    Zg[:, :, b0 + 34:b0 + 37], tpu, 2.0)
                amu = amb[:, lo:hi].unsqueeze(2)
                nc.vector.tensor_mul(Zg[:, :, b0 + 38:b0 + 39], amu, amu)

            # ---------------- frame (W) features, fp32 ------------------
            Wslab = consts.tile([128, KS * NFB], F32)   # col = 64*t + r
            # bf16 slab duplicates each block into both 64-row slots of
            # its group: col = 128*g + 64*s + r, block fb = g, s in {0,1}
            Wslab_b = consts.tile([128, 128 * NFB], BF16)
            nc.vector.memset(Wslab[:], 0.0)
            nc.vector.memset(Wslab_b[:], 0.0)

            W4 = Wslab[:].rearrange("p (t r) -> p r t", r=KS)
            R9 = fr_ap[:, 0:27].rearrange("p (c t) -> p c t", c=9)
            T9 = fr_ap[:, 27:54].rearrange("p (c t) -> p c t", c=9)
            pt3 = ptb.rearrange("p (c t) -> p c t", c=3)
            tt3 = ttb.rearrange("p (c t) -> p c t", c=3)

            m54 = consts.tile([128, 54], F32)
            m54b = consts.tile([128, 54], F32)
            m54c = consts.tile([128, 54], F32)
            mA = consts.tile([128, 18], F32)     # [i, q, t]: P.pt | M^T.pt
            mB = consts.tile([128, 18], F32)     # [i, q, t]: M.tt | T.tt
            t18 = consts.tile([128, 18], F32)
            t9 = consts.tile([128, 9], F32)
            t9b = consts.tile([128, 9], F32)
            t3 = consts.tile([128, 3], F32)

            def w_feats():
                # grams: P (rows 0:9) = R R^T, M (10:19) = R T^T,
                # T (20:29) = T T^T   (ops capped at 3 free dims)
                ma3 = m54[:, 0:27].rearrange("p (i j t) -> p i j t", i=3, j=3)
                mb3 = m54b[:, 0:27].rearrange("p (i j t) -> p i j t", i=3, j=3)
                mc3 = m54c[:, 0:27].rearrange("p (i j t) -> p i j t", i=3, j=3)
                Ri = R9.rearrange("p (i k) t -> p i k t", i=3)
                Ti = T9.rearrange("p (j k) t -> p j k t", j=3)

                def gram(rows, A4, B4):
                    out = W4[:, rows:rows + 9, :].rearrange(
                        "p (i j) t -> p i j t", i=3)
                    a = lambda k: _bc(A4[:, :, k, :], 2, 3)
                    b = lambda k: _bc(B4[:, :, k, :], 1, 3)
                    nc.vector.tensor_mul(ma3, a(0), b(0))
                    nc.vector.tensor_mul(mb3, a(1), b(1))
                    nc.vector.tensor_mul(mc3, a(2), b(2))
                    nc.vector.tensor_add(ma3, ma3, mb3)
                    nc.vector.tensor_add(out, ma3, mc3)

                gram(0, Ri, Ri)
                gram(10, Ri, Ti)
                gram(20, Ti, Ti)

                # matvec pair A: out[i,q,t] = sum_j W[10q+3j+i] * pt_j
                #   q=0: P.pt (P sym, read P_ji); q=1: M^T.pt (read M_ji)
                mv = m54[:].rearrange("p (j i q t) -> p j i q t", j=3, i=3, q=2)
                WqA = W4[:, 0:20, :].rearrange(
                    "p (q h) t -> p q h t", q=2)[:, :, 0:9, :].rearrange(
                    "p q (j i) t -> p j i q t", j=3)
                mAv = mA[:].rearrange("p (i q t) -> p i q t", i=3, q=2)
                ptj = _bc(pt3, 2, 3)                           # [p,j,i,t]
                nc.vector.tensor_mul(mv[:, :, :, 0, :], WqA[:, :, :, 0, :], ptj)
                nc.vector.tensor_mul(mv[:, :, :, 1, :], WqA[:, :, :, 1, :], ptj)
                nc.vector.tensor_add(mAv, mv[:, 0], mv[:, 1])
                nc.vector.tensor_add(mAv, mAv, mv[:, 2])

                # matvec pair B: out[i,q,t] = sum_j W[10+10q+3i+j] * tt_j
                #   q=0: M.tt; q=1: T.tt (T sym)
                WqB = W4[:, 10:30, :].rearrange(
                    "p (q h) t -> p q h t", q=2)[:, :, 0:9, :].rearrange(
                    "p q (i j) t -> p j i q t", i=3)
                mBv = mB[:].rearrange("p (i q t) -> p i q t", i=3, q=2)
                ttj = _bc(tt3, 2, 3)
                nc.vector.tensor_mul(mv[:, :, :, 0, :], WqB[:, :, :, 0, :], ttj)
                nc.vector.tensor_mul(mv[:, :, :, 1, :], WqB[:, :, :, 1, :], ttj)
                nc.vector.tensor_add(mBv, mv[:, 0], mv[:, 1])
                nc.vector.tensor_add(mBv, mBv, mv[:, 2])

                # rows 30:33 = M.tt - P.pt ; rows 34:37 = M^T.pt - T.tt
                nc.vector.tensor_sub(
                    W4[:, 30:33, :], mBv[:, :, 0, :], mAv[:, :, 0, :])
                nc.vector.tensor_sub(
                    W4[:, 34:37, :], mAv[:, :, 1, :], mBv[:, :, 1, :])

                # cf row 38 = pt.(P pt) - 2 pt.(M tt) + tt.(T tt) + EPS
                ptt = fr_ap[:, 54:72].rearrange(
                    "p (q i t) -> p i q t", q=2, i=3)          # pt | tt
                mBm = t18[:].rearrange("p (i q t) -> p i q t", i=3, q=2)
                nc.vector.tensor_mul(mBm, mBv, ptt)  # [pt.Mtt_i | tt.Ttt_i]
                mAm = t9[:].rearrange("p (i t) -> p i t", i=3)
                nc.vector.tensor_mul(mAm, mAv[:, :, 0, :], ptt[:, :, 0, :])
                t9v = t9b[:].rearrange("p (i t) -> p i t", i=3)
                nc.vector.scalar_tensor_tensor(
                    out=t9v, in0=mBm[:, :, 0, :], scalar=-2.0, in1=mAm,
                    op0=OP.mult, op1=OP.add)
                nc.vector.tensor_add(t9v, t9v, mBm[:, :, 1, :])
                nc.vector.tensor_add(t3[:], t9b[:, 0:3], t9b[:, 3:6])
                nc.vector.scalar_tensor_tensor(
                    out=W4[:, 38, :], in0=t3[:], scalar=EPS, in1=t9v[:, 2, :],
                    op0=OP.add, op1=OP.add)

            z_feats(0)
            z_feats(1)
            w_feats()
            # cast rows 0:39, duplicated into both slots (rest stays 0)
            Wb4 = Wslab_b[:].rearrange("p (g s r) -> p g s r", s=2, r=KS)
            Wf4 = Wslab[:].rearrange("p (t r) -> p t r", r=KS)
            nc.vector.tensor_copy(
                Wb4[:, :, :, 0:K], _bc(Wf4[:, :, 0:K], 2, 2))

            # ---------------- transposes (PE + ACT copies) --------------
            identity = consts.tile([128, 128], BF16)
            nc.scalar.copy(identity[:], ident)
            WTi = consts.tile([128, 128 * NFB], BF16)
            ZTi = consts.tile([128, 128 * GZ], BF16)
            with tc.tile_pool(name="pst", bufs=2, space="PSUM") as pst_pool:
                def transpose3(dst, src, glo, ghi):
                    pst = pst_pool.tile([128, 384], BF16, tag="tp")
                    for g in range(glo, ghi):
                        nc.tensor.transpose(
                            pst[:, 128 * (g - glo):128 * (g - glo + 1)],
                            src[:, 128 * g:128 * (g + 1)], identity[:])
                    nw = 128 * (ghi - glo)
                    nc.scalar.copy(dst[:, 128 * glo:128 * ghi], pst[:, 0:nw])

                for pz in range(7):
                    transpose3(ZTi[:], Zslab[:], 3 * pz, 3 * pz + 3)
                transpose3(WTi[:], Wslab_b[:], 0, NFB)

            # ---------------- main loop ---------------------------------
            with (
                tc.tile_pool(name="psm", bufs=2, space="PSUM") as psm_pool,
                tc.tile_pool(name="pso", bufs=1, space="PSUM") as pso_pool,
            ):
                NCOL = 1 + len(FUSED)
                colacc = consts.tile([128, NCOL], F32)
                nc.vector.memset(colacc[:], 0.0)
                ones_b = consts.tile([128, 1], BF16)
                nc.vector.memset(ones_b[:], 1.0)
                ones_ps = pso_pool.tile([2, 512], F32, tag="ones")

                pend = []
                n_ones = [0]
                N_ONES_TOTAL = (NT - len(FUSED)) * ((CHA + 511) // 512)

                def flush_ones(n):
                    while len(pend) > n:
                        dt = pend.pop(0)
                        for mlo in range(0, CHA, 512):
                            mhi = min(mlo + 512, CHA)
                            n_ones[0] += 1
                            nc.tensor.matmul(
                                ones_ps[0:1, 0:mhi - mlo], ones_b[:],
                                dt[:, mlo:mhi],
                                start=(n_ones[0] == 1),
                                stop=(n_ones[0] == N_ONES_TOTAL))

                idx = 0
                nfcol = 1
                for c in range(NCH):
                    u = c // 2
                    alo = CHA * (c % 2)
                    mv_ap = ZTi[64 * u:64 * u + K, alo:alo + CHA]
                    for fb in range(NFB):
                        st_ap = WTi[64 * u:64 * u + K,
                                    128 * fb:128 * fb + 128]
                        ps = psm_pool.tile([128, CHA], F32, tag="main")
                        for mlo in range(0, CHA, 512):
                            mhi = min(mlo + 512, CHA)
                            nc.tensor.matmul(
                                ps[:, mlo:mhi], st_ap, mv_ap[:, mlo:mhi])
                        s = sbuf_s.tile([128, CHA], BF16)
                        nc.scalar.activation(
                            out=s[:], in_=ps[:], func=ACTF.Sqrt)
                        d = sbuf_s.tile([128, CHA], BF16, tag="dmin")
                        if idx in FUSED:
                            nc.vector.tensor_scalar(
                                out=d[:], in0=s[:], scalar1=CLAMP,
                                scalar2=None, op0=OP.min, op1=OP.add,
                                accum_out=colacc[:, nfcol:nfcol + 1])
                            nfcol += 1
                        else:
                            nc.vector.tensor_scalar(
                                out=d[:], in0=s[:], scalar1=CLAMP,
                                scalar2=None, op0=OP.min)
                            pend.append(d)
                        flush_ones(ONES_DELAY)
                        idx += 1
                flush_ones(0)
                # reduce the ones accumulator [1,512] into colacc col 0
                nc.vector.reduce_sum(
                    out=colacc[0:1, 0:1], in_=ones_ps[0:1, :], axis=AX.X)

                # ---------------- epilogue ------------------------------
                ScMc = consts.tile([128, 2], F32)
                nc.vector.reduce_sum(
                    out=ScMc[:, 0:1], in_=colacc[:], axis=AX.X)
                nc.vector.reduce_sum(out=ScMc[:, 1:2], in_=amb, axis=AX.X)
                ones_f = consts.tile([128, 1], F32)
                nc.vector.memset(ones_f[:], 1.0)
                nc.tensor.matmul(ones_ps[0:2, 0:1], ScMc[:], ones_f[:])
                res = consts.tile([2, 1], F32)
                nc.vector.tensor_copy(res[:], ones_ps[0:2, 0:1])
                nc.sync.dma_start(out=out_d[:], in_=res[:])

    nc.compile()
    return nc


def prep_in_maps(inputs):
    """Full (unsharded) numpy inputs -> per-core input dicts.

    fr: per-core frame slice, [128, 72] f32, col = comp*3 + fb where the
        local frame index is 128*fb + p.
    at: atoms replicated, [128, 422] bf16, col = comp*42 + t (t = a//128,
        p = a%128), then mask [42], then a 128x128 identity.
    """
    import ml_dtypes
    f32 = np.float32
    bf16 = ml_dtypes.bfloat16

    def fr_c(x, comps, c):
        a = np.asarray(x, f32).reshape(NF, comps)[NFC * c:NFC * (c + 1)]
        return np.ascontiguousarray(
            a.reshape(NFB, 128, comps).transpose(1, 2, 0)).reshape(128, -1)

    def at_full(x, comps):
        a = np.asarray(x, f32).reshape(NA, comps)
        return np.ascontiguousarray(
            a.reshape(NAB, 128, comps).transpose(1, 2, 0)).reshape(128, -1)

    pp = at_full(inputs["predicted_atom_positions"], 3)
    tp = at_full(inputs["true_atom_positions"], 3)
    am = np.ascontiguousarray(
        np.asarray(inputs["atom_mask"], f32).reshape(NAB, 128).T)
    at = np.concatenate(
        [pp, tp, am, np.eye(128, dtype=f32)], axis=1).astype(bf16)

    in_maps = []
    for c in range(NCORES):
        fr = np.ascontiguousarray(np.concatenate([
            fr_c(inputs["predicted_frames_R"], 9, c),
            fr_c(inputs["true_frames_R"], 9, c),
            fr_c(inputs["predicted_frames_t"], 3, c),
            fr_c(inputs["true_frames_t"], 3, c),
        ], axis=1))
        in_maps.append({"fr": fr, "at": at})
    return in_maps


_NC_CACHE = None


def _get_nc():
    global _NC_CACHE
    if _NC_CACHE is None:
        _NC_CACHE = build_nc()
    return _NC_CACHE


def kernel(**inputs):
    nc = _get_nc()
    in_maps = prep_in_maps(inputs)
    r = run_bass_kernel_spmd(nc, in_maps, core_ids=list(range(NCORES)))
    S = np.float64(0.0)
    M = np.float64(0.0)
    for i in range(NCORES):
        S += np.float64(r.results[i]["out"][0, 0])
        M = np.float64(r.results[i]["out"][1, 0])
    total = S * CNORM / (EPS + M)
    return np.array([total], dtype=np.float32)


# revision 19
# speedup vs baseline: 1.3291x; 1.2042x over previous
"""AllAtomFAPE loss kernel for Trainium2 (8 NeuronCores, SPMD) — v2.

Problem: b=1, N=384 res, F=8 frames/res -> NF=3072 frames; A=14 atoms/res
-> NA=5376 atoms. Output: scalar masked clamped FAPE.

Algorithm (factorized pairwise distance):
  With P = pR pR^T, T = tR tR^T, M = pR tR^T (per frame, 3x3),
    d2(f,a) = (pp-pt)^T P (pp-pt) + (tp-tt)^T T (tp-tt)
              - 2 (pp-pt)^T M (tp-tt)
  expands into a K=39 dot product (rows padded to even offsets so the
  bf16 atom-feature writes stay 4B-aligned; pad rows are zero on both
  sides) between frame features W[:,f] and atom features Z[:,a]:
    rows 0-8   : P[i,j]          <->  pp_i pp_j
    rows 10-18 : M[i,j]          <->  -2 pp_i tp_j
    rows 20-28 : T[i,j]          <->  tp_i tp_j
    rows 30-32 : (M tt - P pt)   <->  2 pp
    rows 34-36 : (M^T pt - T tt) <->  2 tp
    row  38    : c_f + EPS       <->  m_a^2   (mask + eps folded in, so
                                      PSUM d2 = m^2 (d2_true + eps))
  ScalarE computes d = sqrt(psum) with no scale/bias, DVE clamps with a
  constant min(d, 10) (exact for binary masks: m=0 -> d=0), and sums go
  through PE ones-matmuls (most tiles) or the fused DVE min+accum.

Sharding: FRAMES sharded across the 8 cores (384 frames = 3 partition
blocks each, no padding); atoms replicated (5376 = 42 blocks as the
moving free dim). Each core emits [S_partial, mask_sum]; the host does
S_total * CNORM / (eps + mask_sum).

Layouts: PE-transposed via identity matmuls.  W slab holds each frame
block duplicated into both 64-row slots of its group (col = 128*g +
64*s + r) so a stationary exists at base partition 0 AND 64; the
stationary for (fb, u) is WTi[64*u + (0:39), 128*fb + (0:128)].
Atom slab uses pairing (g, g+21): col = 128*g + 64*u + r with block
t = g + 21*u, so ZTi[64*u + (0:39), :] is a contiguous [39, 2688]
moving operand for atom half u.
"""

import numpy as np

import concourse.bacc as bacc
import concourse.bass as bass
import concourse.tile as tile
from concourse import mybir
from concourse.bass_utils import run_bass_kernel_spmd

F32 = mybir.dt.float32
BF16 = mybir.dt.bfloat16
AX = mybir.AxisListType
OP = mybir.AluOpType
ACTF = mybir.ActivationFunctionType

NCORES = 8
NF = 3072
NFC = NF // NCORES     # 384 frames per core
NFB = NFC // 128       # 3 frame blocks
NA = 5376
NAB = NA // 128        # 42 atom blocks
GZ = NAB // 2          # 21 atom groups
K = 39                 # contract rows (with pads)
KS = 64
CHA = 1344             # atom cols per main tile (4 tiles per frame block)
NCH = NA // CHA        # 4
NT = NFB * NCH         # 12 main tiles
EPS = 1e-4
CLAMP = 10.0
ZSCALE = 10.0
CNORM = float(1.0 / (ZSCALE * (3072.0 + EPS)))

# per-tile clamp+sum: tensor_tensor_reduce min-vs-10s-tile with fused
# accumulate (TT form can run the 2x bf16 DVE mode; tensor_scalar+accum
# is stuck at 1x).
GP_TILES = ()


def _bc(ap, dim, n):
    """Broadcast AP along a new axis at position `dim` (stride-0), n copies."""
    return ap.unsqueeze(dim).to_broadcast(
        tuple(ap.shape[:dim]) + (n,) + tuple(ap.shape[dim:])
    )


def build_nc():
    nc = bacc.Bacc(None)

    FRW = 9 * NFB * 2 + 3 * NFB * 2          # 72 cols fp32
    ATW = 3 * NAB * 2 + NAB + 128            # 422 cols bf16 (incl identity)
    fr_d = nc.declare_dram_parameter("fr", [128, FRW], F32, isOutput=False)
    at_d = nc.declare_dram_parameter("at", [128, ATW], BF16, isOutput=False)
    out_d = nc.declare_dram_parameter("out", [2, 1], F32, isOutput=True)

    with tile.TileContext(nc) as tc:
        with (
            tc.tile_pool(name="consts", bufs=1) as consts,
            tc.tile_pool(name="sbuf_s", bufs=3) as sbuf_s,
        ):
            # ---------------- input DMAs --------------------------------
            ats = consts.tile([128, ATW], BF16)
            frs = consts.tile([128, FRW], F32)
            nc.sync.dma_start(out=ats[:, 0:294], in_=at_d[:, 0:294])
            nc.gpsimd.dma_start(out=frs[:], in_=fr_d[:])
            nc.scalar.dma_start(out=ats[:, 294:422], in_=at_d[:, 294:422])
            # touch Sqrt immediately so its ACT table loads during the
            # DMA wait instead of right before the first main-loop sqrt
            tbl = consts.tile([1, 1], F32)
            nc.vector.memset(tbl[:], 0.0)
            nc.scalar.activation(out=tbl[:], in_=tbl[:], func=ACTF.Sqrt)
            at_ap = ats[:]
            ppb = at_ap[:, 0:126]                  # col = c*42 + t
            tpb = at_ap[:, 126:252]
            amb = at_ap[:, 252:294]
            ident = at_ap[:, 294:422]
            fr_ap = frs[:]
            ptb = fr_ap[:, 54:63]                  # col = 54 + 3c + t
            ttb = fr_ap[:, 63:72]

            # ---------------- atom (Z) features, bf16 -------------------
            Zslab = consts.tile([128, 128 * GZ], BF16)
            nc.vector.memset(Zslab[:], 0.0)
            n2pp = consts.tile([128, 126], BF16)
            nc.vector.tensor_scalar_mul(n2pp[:], ppb, -2.0)

            pp3 = ppb.rearrange("p (c t) -> p c t", c=3)       # [128,3,42]
            tp3 = tpb.rearrange("p (c t) -> p c t", c=3)
            n2pp3 = n2pp[:].rearrange("p (c t) -> p c t", c=3)
            Zg = Zslab[:].rearrange("p (g v) -> p g v", v=128)  # [128,21,128]

            def z_feats(u):
                lo, hi = GZ * u, GZ * u + GZ
                b0 = 64 * u
                # [p, g, c] operand views (transpose puts g before c)
                ppu = pp3[:, :, lo:hi].transpose([0, 2, 1])
                tpu = tp3[:, :, lo:hi].transpose([0, 2, 1])
                n2u = n2pp3[:, :, lo:hi].transpose([0, 2, 1])

                def quad(r0, av, bv):
                    out = Zg[:, :, b0 + r0:b0 + r0 + 9].rearrange(
                        "p g (i j) -> p g i j", i=3)
                    nc.vector.tensor_mul(out, _bc(av, 3, 3), _bc(bv, 2, 3))

                quad(0, ppu, ppu)     # pp_i pp_j
                quad(10, n2u, tpu)    # -2 pp_i tp_j
                quad(20, tpu, tpu)    # tp_i tp_j
                nc.vector.tensor_scalar_mul(
                    Zg[:, :, b0 + 30:b0 + 33], ppu, 2.0)
                nc.vector.tensor_scalar_mul(
                    Zg[:, :, b0 + 34:b0 + 37], tpu, 2.0)
                amu = amb[:, lo:hi].unsqueeze(2)
                nc.vector.tensor_mul(Zg[:, :, b0 + 38:b0 + 39], amu, amu)

            # ---------------- frame (W) features, fp32 ------------------
            Wslab = consts.tile([128, KS * NFB], F32)   # col = 64*t + r
            # bf16 slab duplicates each block into both 64-row slots of
            # its group: col = 128*g + 64*s + r, block fb = g, s in {0,1}
            Wslab_b = consts.tile([128, 128 * NFB], BF16)
            nc.vector.memset(Wslab[:], 0.0)
            nc.vector.memset(Wslab_b[:], 0.0)

            W4 = Wslab[:].rearrange("p (t r) -> p r t", r=KS)
            R9 = fr_ap[:, 0:27].rearrange("p (c t) -> p c t", c=9)
            T9 = fr_ap[:, 27:54].rearrange("p (c t) -> p c t", c=9)
            pt3 = ptb.rearrange("p (c t) -> p c t", c=3)
            tt3 = ttb.rearrange("p (c t) -> p c t", c=3)

            m54 = consts.tile([128, 54], F32)
            m54b = consts.tile([128, 54], F32)
            m54c = consts.tile([128, 54], F32)
            mA = consts.tile([128, 18], F32)     # [i, q, t]: P.pt | M^T.pt
            mB = consts.tile([128, 18], F32)     # [i, q, t]: M.tt | T.tt
            t18 = consts.tile([128, 18], F32)
            t9 = consts.tile([128, 9], F32)
            t9b = consts.tile([128, 9], F32)
            t3 = consts.tile([128, 3], F32)

            def w_feats():
                # grams: P (rows 0:9) = R R^T, M (10:19) = R T^T,
                # T (20:29) = T T^T   (ops capped at 3 free dims)
                ma3 = m54[:, 0:27].rearrange("p (i j t) -> p i j t", i=3, j=3)
                mb3 = m54b[:, 0:27].rearrange("p (i j t) -> p i j t", i=3, j=3)
                mc3 = m54c[:, 0:27].rearrange("p (i j t) -> p i j t", i=3, j=3)
                Ri = R9.rearrange("p (i k) t -> p i k t", i=3)
                Ti = T9.rearrange("p (j k) t -> p j k t", j=3)

                def gram(rows, A4, B4):
                    out = W4[:, rows:rows + 9, :].rearrange(
                        "p (i j) t -> p i j t", i=3)
                    a = lambda k: _bc(A4[:, :, k, :], 2, 3)
                    b = lambda k: _bc(B4[:, :, k, :], 1, 3)
                    nc.vector.tensor_mul(ma3, a(0), b(0))
                    nc.vector.tensor_mul(mb3, a(1), b(1))
                    nc.vector.tensor_mul(mc3, a(2), b(2))
                    nc.vector.tensor_add(ma3, ma3, mb3)
                    nc.vector.tensor_add(out, ma3, mc3)

                gram(0, Ri, Ri)
                gram(10, Ri, Ti)
                gram(20, Ti, Ti)

                # matvec pair A: out[i,q,t] = sum_j W[10q+3j+i] * pt_j
                #   q=0: P.pt (P sym, read P_ji); q=1: M^T.pt (read M_ji)
                mv = m54[:].rearrange("p (j i q t) -> p j i q t", j=3, i=3, q=2)
                WqA = W4[:, 0:20, :].rearrange(
                    "p (q h) t -> p q h t", q=2)[:, :, 0:9, :].rearrange(
                    "p q (j i) t -> p j i q t", j=3)
                mAv = mA[:].rearrange("p (i q t) -> p i q t", i=3, q=2)
                ptj = _bc(pt3, 2, 3)                           # [p,j,i,t]
                nc.vector.tensor_mul(mv[:, :, :, 0, :], WqA[:, :, :, 0, :], ptj)
                nc.vector.tensor_mul(mv[:, :, :, 1, :], WqA[:, :, :, 1, :], ptj)
                nc.vector.tensor_add(mAv, mv[:, 0], mv[:, 1])
                nc.vector.tensor_add(mAv, mAv, mv[:, 2])

                # matvec pair B: out[i,q,t] = sum_j W[10+10q+3i+j] * tt_j
                #   q=0: M.tt; q=1: T.tt (T sym)
                WqB = W4[:, 10:30, :].rearrange(
                    "p (q h) t -> p q h t", q=2)[:, :, 0:9, :].rearrange(
                    "p q (i j) t -> p j i q t", i=3)
                mBv = mB[:].rearrange("p (i q t) -> p i q t", i=3, q=2)
                ttj = _bc(tt3, 2, 3)
                nc.vector.tensor_mul(mv[:, :, :, 0, :], WqB[:, :, :, 0, :], ttj)
                nc.vector.tensor_mul(mv[:, :, :, 1, :], WqB[:, :, :, 1, :], ttj)
                nc.vector.tensor_add(mBv, mv[:, 0], mv[:, 1])
                nc.vector.tensor_add(mBv, mBv, mv[:, 2])

                # rows 30:33 = M.tt - P.pt ; rows 34:37 = M^T.pt - T.tt
                nc.vector.tensor_sub(
                    W4[:, 30:33, :], mBv[:, :, 0, :], mAv[:, :, 0, :])
                nc.vector.tensor_sub(
                    W4[:, 34:37, :], mAv[:, :, 1, :], mBv[:, :, 1, :])

                # cf row 38 = pt.(P pt) - 2 pt.(M tt) + tt.(T tt) + EPS
                ptt = fr_ap[:, 54:72].rearrange(
                    "p (q i t) -> p i q t", q=2, i=3)          # pt | tt
                mBm = t18[:].rearrange("p (i q t) -> p i q t", i=3, q=2)
                nc.vector.tensor_mul(mBm, mBv, ptt)  # [pt.Mtt_i | tt.Ttt_i]
                mAm = t9[:].rearrange("p (i t) -> p i t", i=3)
                nc.vector.tensor_mul(mAm, mAv[:, :, 0, :], ptt[:, :, 0, :])
                t9v = t9b[:].rearrange("p (i t) -> p i t", i=3)
                nc.vector.scalar_tensor_tensor(
                    out=t9v, in0=mBm[:, :, 0, :], scalar=-2.0, in1=mAm,
                    op0=OP.mult, op1=OP.add)
                nc.vector.tensor_add(t9v, t9v, mBm[:, :, 1, :])
                nc.vector.tensor_add(t3[:], t9b[:, 0:3], t9b[:, 3:6])
                nc.vector.scalar_tensor_tensor(
                    out=W4[:, 38, :], in0=t3[:], scalar=EPS, in1=t9v[:, 2, :],
                    op0=OP.add, op1=OP.add)

            z_feats(0)
            z_feats(1)
            w_feats()
            # cast rows 0:39, duplicated into both slots (rest stays 0)
            Wb4 = Wslab_b[:].rearrange("p (g s r) -> p g s r", s=2, r=KS)
            Wf4 = Wslab[:].rearrange("p (t r) -> p t r", r=KS)
            nc.vector.tensor_copy(
                Wb4[:, :, :, 0:K], _bc(Wf4[:, :, 0:K], 2, 2))

            # ---------------- transposes (PE + ACT copies) --------------
            identity = consts.tile([128, 128], BF16)
            nc.scalar.copy(identity[:], ident)
            WTi = consts.tile([128, 128 * NFB], BF16)
            ZTi = consts.tile([128, 128 * GZ], BF16)
            with tc.tile_pool(name="pst", bufs=2, space="PSUM") as pst_pool:
                def transpose3(dst, src, glo, ghi):
                    pst = pst_pool.tile([128, 384], BF16, tag="tp")
                    for g in range(glo, ghi):
                        nc.tensor.transpose(
                            pst[:, 128 * (g - glo):128 * (g - glo + 1)],
                            src[:, 128 * g:128 * (g + 1)], identity[:])
                    nw = 128 * (ghi - glo)
                    nc.scalar.copy(dst[:, 128 * glo:128 * ghi], pst[:, 0:nw])

                for pz in range(7):
                    transpose3(ZTi[:], Zslab[:], 3 * pz, 3 * pz + 3)
                transpose3(WTi[:], Wslab_b[:], 0, NFB)

            # ---------------- main loop ---------------------------------
            with (
                tc.tile_pool(name="psm", bufs=2, space="PSUM") as psm_pool,
                tc.tile_pool(name="pso", bufs=1, space="PSUM") as pso_pool,
            ):
                colacc = consts.tile([128, NT], F32)
                nc.vector.memset(colacc[:], 0.0)
                tens10 = consts.tile([128, CHA], BF16)
                nc.vector.memset(tens10[:], CLAMP)

                idx = 0
                for c in range(NCH):
                    u = c // 2
                    alo = CHA * (c % 2)
                    mv_ap = ZTi[64 * u:64 * u + K, alo:alo + CHA]
                    for fb in range(NFB):
                        st_ap = WTi[64 * u:64 * u + K,
                                    128 * fb:128 * fb + 128]
                        ps = psm_pool.tile([128, CHA], F32, tag="main")
                        for mlo in range(0, CHA, 512):
                            mhi = min(mlo + 512, CHA)
                            nc.tensor.matmul(
                                ps[:, mlo:mhi], st_ap, mv_ap[:, mlo:mhi])
                        s = sbuf_s.tile([128, CHA], BF16)
                        nc.scalar.activation(
                            out=s[:], in_=ps[:], func=ACTF.Sqrt)
                        d = sbuf_s.tile([128, CHA], BF16, tag="dmin")
                        nc.vector.tensor_scalar(
                            out=d[:], in0=s[:], scalar1=CLAMP,
                            scalar2=None, op0=OP.min, op1=OP.add,
                            accum_out=colacc[:, idx:idx + 1])
                        idx += 1

                # ---------------- epilogue ------------------------------
                ScMc = consts.tile([128, 2], F32)
                nc.vector.reduce_sum(
                    out=ScMc[:, 0:1], in_=colacc[:], axis=AX.X)
                nc.vector.reduce_sum(out=ScMc[:, 1:2], in_=amb, axis=AX.X)
                ones_f = consts.tile([128, 1], F32)
                nc.vector.memset(ones_f[:], 1.0)
                psfin = pso_pool.tile([2, 1], F32, tag="fin")
                nc.tensor.matmul(psfin[:], ScMc[:], ones_f[:])
                res = consts.tile([2, 1], F32)
                nc.vector.tensor_copy(res[:], psfin[:])
                nc.sync.dma_start(out=out_d[:], in_=res[:])

    nc.compile()
    return nc


def prep_in_maps(inputs):
    """Full (unsharded) numpy inputs -> per-core input dicts.

    fr: per-core frame slice, [128, 72] f32, col = comp*3 + fb where the
        local frame index is 128*fb + p.
    at: atoms replicated, [128, 422] bf16, col = comp*42 + t (t = a//128,
        p = a%128), then mask [42], then a 128x128 identity.
    """
    import ml_dtypes
    f32 = np.float32
    bf16 = ml_dtypes.bfloat16

    def fr_c(x, comps, c):
        a = np.asarray(x, f32).reshape(NF, comps)[NFC * c:NFC * (c + 1)]
        return np.ascontiguousarray(
            a.reshape(NFB, 128, comps).transpose(1, 2, 0)).reshape(128, -1)

    def at_full(x, comps):
        a = np.asarray(x, f32).reshape(NA, comps)
        return np.ascontiguousarray(
            a.reshape(NAB, 128, comps).transpose(1, 2, 0)).reshape(128, -1)

    pp = at_full(inputs["predicted_atom_positions"], 3)
    tp = at_full(inputs["true_atom_positions"], 3)
    am = np.ascontiguousarray(
        np.asarray(inputs["atom_mask"], f32).reshape(NAB, 128).T)
    at = np.concatenate(
        [pp, tp, am, np.eye(128, dtype=f32)], axis=1).astype(bf16)

    in_maps = []
    for c in range(NCORES):
        fr = np.ascontiguousarray(np.concatenate([
            fr_c(inputs["predicted_frames_R"], 9, c),
            fr_c(inputs["true_frames_R"], 9, c),
            fr_c(inputs["predicted_frames_t"], 3, c),
            fr_c(inputs["true_frames_t"], 3, c),
        ], axis=1))
        in_maps.append({"fr": fr, "at": at})
    return in_maps


_NC_CACHE = None


def _get_nc():
    global _NC_CACHE
    if _NC_CACHE is None:
        _NC_CACHE = build_nc()
    return _NC_CACHE


def kernel(**inputs):
    nc = _get_nc()
    in_maps = prep_in_maps(inputs)
    r = run_bass_kernel_spmd(nc, in_maps, core_ids=list(range(NCORES)))
    S = np.float64(0.0)
    M = np.float64(0.0)
    for i in range(NCORES):
        S += np.float64(r.results[i]["out"][0, 0])
        M = np.float64(r.results[i]["out"][1, 0])
    total = S * CNORM / (EPS + M)
    return np.array([total], dtype=np.float32)
